# revision 15
# baseline (speedup 1.0000x reference)
"""MoE routing kernel for Trainium2, 8 NeuronCores.

Strategy (expert-parallel, one device launch):
  Host: gating softmax + top-k in float64 (0.8% of total FLOPs;
  selection is exact vs the f32 reference since top-k margins are
  orders of magnitude above f32 rounding noise). From the routing,
  build per-expert token lists, pre-scale each gathered token by its
  gate probability (experts are linear, so scaling inputs is exactly
  scaling outputs), transpose, and cast to bf16.
  Device (expert-parallel): each core runs its E/8 experts' matmuls
  in bf16 (full-rate PE, fp32 PSUM accumulation) with k-outer wave
  scheduling so the PE streams behind the DMA, and writes fp32
  outputs. All expert FLOPs and all bulk HBM traffic are on device.
  Host: scatter-adds the compact per-expert outputs into [B, DOUT].

bf16 inputs halve the HBM traffic that made the fp32 version
DMA-bound (~410 GB/s saturated); the kernel is then PE-bound at
~236 ns per 512-row matmul.
"""
import numpy as np
from contextlib import ExitStack

import ml_dtypes

import concourse.bass as bass
import concourse.mybir as mybir
from concourse import bacc, tile
from concourse.bass_utils import run_bass_kernel_spmd

NCORES = 8
P = 128
F32 = mybir.dt.float32
BF16 = mybir.dt.bfloat16
NPBF16 = ml_dtypes.bfloat16

# test-harness knobs (ignored in normal use)
TRACE = False
LAST_EXEC_NS = []
LAST_RESULTS = {}

_cache = {}


def _warmup_pe(nc, pool, ps_pool, n_mm, tag="ps"):
    """Dummy bf16 matmuls on scratch data, issued at kernel start so the
    PE's HAM clock-gate ramps toward 2.4 GHz while the input DMAs
    stream in. Vector memset so the warmup isn't gated on GpSimd."""
    wt = pool.tile([P, 512], BF16, name="warm_sb")
    nc.vector.memset(wt[:], 1.0)
    wp = ps_pool.tile([P, 512], F32, name="warm_ps", tag=tag)
    for _ in range(n_mm):
        nc.tensor.matmul(wp[:], wt[:, :P], wt[:], start=True, stop=True)
    return wt, wp


def _build_expert(C, DIN, DOUT, EPC):
    """Per-core expert compute: for each of the core's EPC experts,
    y_e = xg_e @ W_e over a capacity-C padded, gate-pre-scaled token
    list. bf16 operands, fp32 PSUM, k-outer in waves of 4 PSUM
    accumulation groups (8 banks, two waves in flight) so the PE
    streams behind the DMA.

    Inputs : xgT  [EPC, DIN, C]    bf16 (gathered tokens * gate value,
                                         transposed)
             wexp [EPC, DIN, DOUT] bf16
    Output : yout [EPC, C, DOUT]   f32
    """
    key = ("exp", C, DIN, DOUT, EPC)
    if key in _cache:
        return _cache[key]
    KT = DIN // P
    MT = C // P
    NF = 512
    assert DOUT % NF == 0
    NT = DOUT // NF
    nc = bacc.Bacc("TRN2", target_bir_lowering=False, debug=False,
                   num_devices=NCORES)
    xgT = nc.dram_tensor("xgT", [EPC, DIN, C], BF16, kind="ExternalInput")
    wexp = nc.dram_tensor("wexp", [EPC, DIN, DOUT], BF16,
                          kind="ExternalInput")
    yout = nc.dram_tensor("yout", [EPC, C, DOUT], F32,
                          kind="ExternalOutput")

    with tile.TileContext(nc) as tc:
        with ExitStack() as ctx:
            xg_pool = ctx.enter_context(tc.tile_pool(name="xg", bufs=2))
            w_pool = ctx.enter_context(tc.tile_pool(name="w", bufs=2))
            out_pool = ctx.enter_context(tc.tile_pool(name="out",
                                                      bufs=12))
            ps = ctx.enter_context(tc.tile_pool(name="ps", bufs=8,
                                                space="PSUM"))
            warm_pool = ctx.enter_context(tc.tile_pool(name="warm", bufs=1))
            # warmup PSUM tile shares the wave slots (transient).
            # ~8 warmups cover the ~5us preamble-to-first-chunk window.
            _warmup_pe(nc, warm_pool, ps, 8, tag="ps")

            # Hoist ALL input loads (both experts) to the front of the
            # sync (xg) / scalar (w) queues, ahead of any output store,
            # so expert 1's loads are never stuck behind expert 0's
            # stores in queue order. EPC tiles fit the pools exactly.
            assert EPC <= 2
            xg_ts, w_ts = [], []
            for e in range(EPC):
                xg_t = xg_pool.tile([P, KT, C], BF16, tag="xg",
                                    name=f"xg{e}")
                w_t = w_pool.tile([P, KT, DOUT], BF16, tag="w",
                                  name=f"w{e}")
                xg_ts.append(xg_t)
                w_ts.append(w_t)
                for k in range(KT):
                    lo, hi = k * P, (k + 1) * P
                    if e == 0 and k == 0:
                        # split only the very first chunk so the first
                        # matmuls start sooner after the DMA engines
                        # wake up (more splits would throttle the fill:
                        # descriptor issue costs ~600ns each)
                        half = 4 * P
                        nc.sync.dma_start(xg_t[:, 0, :half],
                                          xgT[0, :P, :half])
                        nc.sync.dma_start(xg_t[:, 0, half:],
                                          xgT[0, :P, half:])
                        nc.scalar.dma_start(w_t[:, 0], wexp[0, :P, :])
                    else:
                        nc.sync.dma_start(xg_t[:, k], xgT[e, lo:hi, :])
                        nc.scalar.dma_start(w_t[:, k], wexp[e, lo:hi, :])

            for e in range(EPC):
                xg_t, w_t = xg_ts[e], w_ts[e]
                # k-outer waves of concurrent PSUM groups, m-major.
                # The first wave of expert 0 takes all 8 banks: its
                # ~14us of matmul work hides the input-DMA fill and
                # keeps the PE busy so the clock ramp completes early.
                groups = [(m, n) for m in range(MT) for n in range(NT)]
                w0 = 0
                first = (e == 0)
                while w0 < len(groups):
                    wave = groups[w0:w0 + (8 if first else 4)]
                    w0 += len(wave)
                    first = False
                    pss = {g: ps.tile([P, NF], F32, tag="ps",
                                      name=f"ps_{e}_{g[0]}_{g[1]}")
                           for g in wave}
                    for k in range(KT):
                        for (m, n) in wave:
                            nc.tensor.matmul(
                                pss[(m, n)][:],
                                xg_t[:, k, m * P:(m + 1) * P],
                                w_t[:, k, n * NF:(n + 1) * NF],
                                start=(k == 0),
                                stop=(k == KT - 1),
                            )
                    # evict each finished group on the Vector engine
                    # and store right away, alternating store queues by
                    # output half so neither backs up
                    for (m, n) in wave:
                        dst = yout[e, m * P:(m + 1) * P,
                                   n * NF:(n + 1) * NF]
                        ot = out_pool.tile([P, NF], F32, tag="out",
                                           name=f"out_{e}_{m}_{n}")
                        nc.vector.tensor_copy(ot[:], pss[(m, n)][:])
                        eng = nc.sync if n == 0 else nc.scalar
                        eng.dma_start(dst, ot[:])
    nc.compile()
    _cache[key] = nc
    return nc


def _run(nc, in_maps):
    kw = {}
    if TRACE:
        kw["trace"] = True
    res = run_bass_kernel_spmd(nc, in_maps, list(range(NCORES)), **kw)
    if TRACE:
        LAST_EXEC_NS.append(res.exec_time_ns)
        LAST_RESULTS["last"] = res
    return res.results


def kernel(x, gate_w, gate_b, expert_w, expert_b, topk):
    x = np.ascontiguousarray(np.asarray(x, dtype=np.float32))
    gate_w = np.asarray(gate_w, dtype=np.float32)
    gate_b = np.asarray(gate_b, dtype=np.float32)
    expert_w = np.asarray(expert_w, dtype=np.float32)
    expert_b = np.asarray(expert_b, dtype=np.float32)
    topk = int(topk)

    B, DIN = x.shape
    E, _, DOUT = expert_w.shape
    assert B % P == 0 and DIN % P == 0
    EPC = E // NCORES
    assert EPC * NCORES == E

    # ---- host: gating (softmax + top-k) in float64 ----
    # Exact relative to the f32 reference: top-k margins (~1e-4 min)
    # dwarf the ~1e-5 f32 summation noise, so selection matches, and
    # the f64 probabilities are tighter than the reference's own f32.
    logits = x.astype(np.float64) @ gate_w.astype(np.float64).T \
        + gate_b.astype(np.float64)
    if topk < E:
        kth = np.partition(logits, E - topk, axis=1)[:, E - topk]
        mask = logits >= kth[:, None]
    else:
        mask = np.ones_like(logits, dtype=bool)
    z = np.exp(logits - logits.max(axis=1, keepdims=True))
    probs = z / z.sum(axis=1, keepdims=True)
    wfull = np.where(mask, probs, 0.0).astype(np.float32)

    # ---- host: routing bookkeeping + gather (pre-scaled, bf16) ----
    toks = [np.nonzero(wfull[:, e])[0] for e in range(E)]
    maxcnt = max(1, max(len(t) for t in toks))
    C = ((maxcnt + P - 1) // P) * P

    nc = _build_expert(C, DIN, DOUT, EPC)
    in_maps = []
    for c in range(NCORES):
        xgT = np.zeros((EPC, DIN, C), NPBF16)
        for j in range(EPC):
            e = EPC * c + j
            t = toks[e]
            xs = x[t] * wfull[t, e][:, None]      # gate-scaled tokens
            xgT[j, :, :len(t)] = xs.T.astype(NPBF16)
        in_maps.append({"xgT": xgT,
                        "wexp": expert_w[EPC * c:EPC * (c + 1)]
                        .astype(NPBF16)})
    r = _run(nc, in_maps)

    # ---- host: scatter-add compact outputs (unshard) ----
    y = np.zeros((B, DOUT), np.float32)
    for c in range(NCORES):
        yo = np.asarray(r[c]["yout"], dtype=np.float32)
        for j in range(EPC):
            e = EPC * c + j
            t = toks[e]
            y[t] += yo[j, :len(t)]
    if np.any(expert_b):
        for e in range(E):
            t = toks[e]
            y[t] += wfull[t, e][:, None] * expert_b[e][None, :]
    return y


# revision 16
# speedup vs baseline: 1.1588x; 1.1588x over previous
"""MoE routing kernel for Trainium2, 8 NeuronCores.

Strategy (expert-parallel, one device launch):
  Host: gating softmax + top-k in float64 (0.8% of total FLOPs;
  selection is exact vs the f32 reference since top-k margins are
  orders of magnitude above f32 rounding noise). From the routing,
  build per-expert token lists, pre-scale each gathered token by its
  gate probability (experts are linear, so scaling inputs is exactly
  scaling outputs), transpose, and cast to bf16.
  Device (expert-parallel): each core runs its E/8 experts' matmuls
  in bf16 (full-rate PE, fp32 PSUM accumulation) with k-outer wave
  scheduling so the PE streams behind the DMA, and writes fp32
  outputs. All expert FLOPs and all bulk HBM traffic are on device.
  Host: scatter-adds the compact per-expert outputs into [B, DOUT].

bf16 inputs halve the HBM traffic that made the fp32 version
DMA-bound (~410 GB/s saturated); the kernel is then PE-bound at
~216 ns per 512-row matmul (the 61.4us PE floor for 2x1152 padded
rows per core), plus ~13us launch preamble/DMA spin-up and ~5us
drain/teardown.
"""
import numpy as np
from contextlib import ExitStack

import ml_dtypes

import concourse.bass as bass
import concourse.mybir as mybir
from concourse import bacc, tile
from concourse.bass_utils import run_bass_kernel_spmd

NCORES = 8
P = 128
F32 = mybir.dt.float32
BF16 = mybir.dt.bfloat16
NPBF16 = ml_dtypes.bfloat16

# test-harness knobs (ignored in normal use)
TRACE = False
LAST_EXEC_NS = []
LAST_RESULTS = {}

_cache = {}


def _warmup_pe(nc, pool, ps_pool, n_mm, tag="ps"):
    """Dummy bf16 matmuls on scratch data, issued at kernel start so the
    PE's HAM clock-gate ramps toward 2.4 GHz while the input DMAs
    stream in. Vector memset so the warmup isn't gated on GpSimd."""
    wt = pool.tile([P, 512], BF16, name="warm_sb")
    nc.vector.memset(wt[:], 1.0)
    wp = ps_pool.tile([P, 512], F32, name="warm_ps", tag=tag)
    for _ in range(n_mm):
        nc.tensor.matmul(wp[:], wt[:, :P], wt[:], start=True, stop=True)
    return wt, wp


def _build_expert(C, DIN, DOUT, EPC):
    """Per-core expert compute: for each of the core's EPC experts,
    y_e = xg_e @ W_e over a capacity-C padded, gate-pre-scaled token
    list. bf16 operands, fp32 PSUM, k-outer in waves of 4 PSUM
    accumulation groups (8 banks, two waves in flight) so the PE
    streams behind the DMA.

    Inputs : xgT  [EPC, DIN, C]    bf16 (gathered tokens * gate value,
                                         transposed)
             wexp [EPC, DIN, DOUT] bf16
    Output : yout [EPC, C, DOUT]   f32
    """
    key = ("exp", C, DIN, DOUT, EPC)
    if key in _cache:
        return _cache[key]
    KT = DIN // P
    MT = C // P
    NF = 512
    assert DOUT % NF == 0
    NT = DOUT // NF
    nc = bacc.Bacc("TRN2", target_bir_lowering=False, debug=False,
                   num_devices=NCORES)
    xgT = nc.dram_tensor("xgT", [EPC, DIN, C], BF16, kind="ExternalInput")
    wexp = nc.dram_tensor("wexp", [EPC, DIN, DOUT], BF16,
                          kind="ExternalInput")
    yout = nc.dram_tensor("yout", [EPC, C, DOUT], F32,
                          kind="ExternalOutput")

    with tile.TileContext(nc) as tc:
        with ExitStack() as ctx:
            xg_pool = ctx.enter_context(tc.tile_pool(name="xg", bufs=2))
            w_pool = ctx.enter_context(tc.tile_pool(name="w", bufs=2))
            out_pool = ctx.enter_context(tc.tile_pool(name="out",
                                                      bufs=12))
            ps = ctx.enter_context(tc.tile_pool(name="ps", bufs=8,
                                                space="PSUM"))
            warm_pool = ctx.enter_context(tc.tile_pool(name="warm", bufs=1))
            # warmup PSUM tile shares the wave slots (transient).
            # ~10 warmups cover the ~6us preamble-to-first-chunk window.
            _warmup_pe(nc, warm_pool, ps, 10, tag="ps")

            # Hoist ALL input loads (both experts) to the front of the
            # sync (xg) / scalar (w) queues, ahead of any output store,
            # so expert 1's loads are never stuck behind expert 0's
            # stores in queue order. EPC tiles fit the pools exactly.
            assert EPC <= 2
            xg_ts, w_ts = [], []
            for e in range(EPC):
                xg_t = xg_pool.tile([P, KT, C], BF16, tag="xg",
                                    name=f"xg{e}")
                w_t = w_pool.tile([P, KT, DOUT], BF16, tag="w",
                                  name=f"w{e}")
                xg_ts.append(xg_t)
                w_ts.append(w_t)
                for k in range(KT):
                    lo, hi = k * P, (k + 1) * P
                    if e == 0 and k == 0:
                        # split only the very first chunk so the first
                        # matmuls start sooner after the DMA engines
                        # wake up (more splits would throttle the fill:
                        # descriptor issue costs ~600ns each)
                        half = 4 * P
                        nc.sync.dma_start(xg_t[:, 0, :half],
                                          xgT[0, :P, :half])
                        nc.sync.dma_start(xg_t[:, 0, half:],
                                          xgT[0, :P, half:])
                        nc.scalar.dma_start(w_t[:, 0], wexp[0, :P, :])
                    else:
                        nc.sync.dma_start(xg_t[:, k], xgT[e, lo:hi, :])
                        nc.scalar.dma_start(w_t[:, k], wexp[e, lo:hi, :])

            for e in range(EPC):
                xg_t, w_t = xg_ts[e], w_ts[e]
                # k-outer waves of concurrent PSUM groups, m-major.
                # The first wave of expert 0 takes all 8 banks: its
                # ~14us of matmul work hides the input-DMA fill and
                # keeps the PE busy so the clock ramp completes early.
                groups = [(m, n) for m in range(MT) for n in range(NT)]
                w0 = 0
                first = (e == 0)
                while w0 < len(groups):
                    wave = groups[w0:w0 + (8 if first else 4)]
                    w0 += len(wave)
                    first = False
                    pss = {g: ps.tile([P, NF], F32, tag="ps",
                                      name=f"ps_{e}_{g[0]}_{g[1]}")
                           for g in wave}
                    for k in range(KT):
                        for (m, n) in wave:
                            nc.tensor.matmul(
                                pss[(m, n)][:],
                                xg_t[:, k, m * P:(m + 1) * P],
                                w_t[:, k, n * NF:(n + 1) * NF],
                                start=(k == 0),
                                stop=(k == KT - 1),
                            )
                    # evict each finished group on the Vector engine
                    # and store right away, alternating store queues by
                    # output half so neither backs up
                    for (m, n) in wave:
                        dst = yout[e, m * P:(m + 1) * P,
                                   n * NF:(n + 1) * NF]
                        ot = out_pool.tile([P, NF], F32, tag="out",
                                           name=f"out_{e}_{m}_{n}")
                        nc.vector.tensor_copy(ot[:], pss[(m, n)][:])
                        eng = nc.sync if n == 0 else nc.scalar
                        eng.dma_start(dst, ot[:])
    nc.compile()
    _cache[key] = nc
    return nc


def _run(nc, in_maps):
    kw = {}
    if TRACE:
        kw["trace"] = True
    res = run_bass_kernel_spmd(nc, in_maps, list(range(NCORES)), **kw)
    if TRACE:
        LAST_EXEC_NS.append(res.exec_time_ns)
        LAST_RESULTS["last"] = res
    return res.results


def kernel(x, gate_w, gate_b, expert_w, expert_b, topk):
    x = np.ascontiguousarray(np.asarray(x, dtype=np.float32))
    gate_w = np.asarray(gate_w, dtype=np.float32)
    gate_b = np.asarray(gate_b, dtype=np.float32)
    expert_w = np.asarray(expert_w, dtype=np.float32)
    expert_b = np.asarray(expert_b, dtype=np.float32)
    topk = int(topk)

    B, DIN = x.shape
    E, _, DOUT = expert_w.shape
    assert B % P == 0 and DIN % P == 0
    EPC = E // NCORES
    assert EPC * NCORES == E

    # ---- host: gating (softmax + top-k) in float64 ----
    # Exact relative to the f32 reference: top-k margins (~1e-4 min)
    # dwarf the ~1e-5 f32 summation noise, so selection matches, and
    # the f64 probabilities are tighter than the reference's own f32.
    logits = x.astype(np.float64) @ gate_w.astype(np.float64).T \
        + gate_b.astype(np.float64)
    if topk < E:
        kth = np.partition(logits, E - topk, axis=1)[:, E - topk]
        mask = logits >= kth[:, None]
    else:
        mask = np.ones_like(logits, dtype=bool)
    z = np.exp(logits - logits.max(axis=1, keepdims=True))
    probs = z / z.sum(axis=1, keepdims=True)
    wfull = np.where(mask, probs, 0.0).astype(np.float32)

    # ---- host: routing bookkeeping + gather (pre-scaled, bf16) ----
    toks = [np.nonzero(wfull[:, e])[0] for e in range(E)]
    maxcnt = max(1, max(len(t) for t in toks))
    C = ((maxcnt + P - 1) // P) * P

    nc = _build_expert(C, DIN, DOUT, EPC)
    in_maps = []
    for c in range(NCORES):
        xgT = np.zeros((EPC, DIN, C), NPBF16)
        for j in range(EPC):
            e = EPC * c + j
            t = toks[e]
            xs = x[t] * wfull[t, e][:, None]      # gate-scaled tokens
            xgT[j, :, :len(t)] = xs.T.astype(NPBF16)
        in_maps.append({"xgT": xgT,
                        "wexp": expert_w[EPC * c:EPC * (c + 1)]
                        .astype(NPBF16)})
    r = _run(nc, in_maps)

    # ---- host: scatter-add compact outputs (unshard) ----
    y = np.zeros((B, DOUT), np.float32)
    for c in range(NCORES):
        yo = np.asarray(r[c]["yout"], dtype=np.float32)
        for j in range(EPC):
            e = EPC * c + j
            t = toks[e]
            y[t] += yo[j, :len(t)]
    if np.any(expert_b):
        for e in range(E):
            t = toks[e]
            y[t] += wfull[t, e][:, None] * expert_b[e][None, :]
    return y


# revision 17
# speedup vs baseline: 1.1729x; 1.0122x over previous
"""MoE routing kernel for Trainium2, 8 NeuronCores.

Strategy (expert-parallel, one device launch):
  Host: gating softmax + top-k in float64 (0.8% of total FLOPs;
  selection is exact vs the f32 reference since top-k margins are
  orders of magnitude above f32 rounding noise). From the routing,
  build per-expert token lists, pre-scale each gathered token by its
  gate probability (experts are linear, so scaling inputs is exactly
  scaling outputs), transpose, and cast to bf16.
  Device (expert-parallel): each core runs its E/8 experts' matmuls
  in bf16 (full-rate PE, fp32 PSUM accumulation) with k-outer wave
  scheduling so the PE streams behind the DMA, and writes fp32
  outputs. All expert FLOPs and all bulk HBM traffic are on device.
  Host: scatter-adds the compact per-expert outputs into [B, DOUT].

bf16 inputs halve the HBM traffic that made the fp32 version
DMA-bound (~410 GB/s saturated); the kernel is then PE-bound at
~216 ns per 512-row matmul (the 61.4us PE floor for 2x1152 padded
rows per core), plus ~13us launch preamble/DMA spin-up and ~5us
drain/teardown.
"""
import numpy as np
from contextlib import ExitStack

import ml_dtypes

import concourse.mybir as mybir
from concourse import bacc, tile
from concourse.bass_utils import run_bass_kernel_spmd

NCORES = 8
P = 128
F32 = mybir.dt.float32
BF16 = mybir.dt.bfloat16
NPBF16 = ml_dtypes.bfloat16

# test-harness knobs (ignored in normal use)
TRACE = False
LAST_EXEC_NS = []
LAST_RESULTS = {}

_cache = {}


def _warmup_pe(nc, pool, ps_pool, n_mm, tag="ps"):
    """Dummy bf16 matmuls on scratch data, issued at kernel start so the
    PE's HAM clock-gate ramps toward 2.4 GHz while the input DMAs
    stream in. Vector memset so the warmup isn't gated on GpSimd."""
    wt = pool.tile([P, 512], BF16, name="warm_sb")
    nc.vector.memset(wt[:], 1.0)
    wp = ps_pool.tile([P, 512], F32, name="warm_ps", tag=tag)
    for _ in range(n_mm):
        nc.tensor.matmul(wp[:], wt[:, :P], wt[:], start=True, stop=True)
    return wt, wp


def _build_expert(C, DIN, DOUT, EPC):
    """Per-core expert compute: for each of the core's EPC experts,
    y_e = xg_e @ W_e over a capacity-C padded, gate-pre-scaled token
    list. bf16 operands, fp32 PSUM, k-outer in waves of 4 PSUM
    accumulation groups (8 banks, two waves in flight) so the PE
    streams behind the DMA.

    Inputs : xgT  [EPC, DIN, C]    bf16 (gathered tokens * gate value,
                                         transposed)
             wexp [EPC, DIN, DOUT] bf16
    Output : yout [EPC, C, DOUT]   f32
    """
    key = ("exp", C, DIN, DOUT, EPC)
    if key in _cache:
        return _cache[key]
    KT = DIN // P
    MT = C // P
    NF = 512
    assert DOUT % NF == 0
    NT = DOUT // NF
    nc = bacc.Bacc("TRN2", target_bir_lowering=False, debug=False,
                   num_devices=NCORES)
    xgT = nc.dram_tensor("xgT", [EPC, DIN, C], BF16, kind="ExternalInput")
    wexp = nc.dram_tensor("wexp", [EPC, DIN, DOUT], BF16,
                          kind="ExternalInput")
    yout = nc.dram_tensor("yout", [EPC, C, DOUT], F32,
                          kind="ExternalOutput")

    with tile.TileContext(nc) as tc:
        with ExitStack() as ctx:
            xg_pool = ctx.enter_context(tc.tile_pool(name="xg", bufs=2))
            w_pool = ctx.enter_context(tc.tile_pool(name="w", bufs=2))
            out_pool = ctx.enter_context(tc.tile_pool(name="out",
                                                      bufs=12))
            ps = ctx.enter_context(tc.tile_pool(name="ps", bufs=8,
                                                space="PSUM"))
            warm_pool = ctx.enter_context(tc.tile_pool(name="warm", bufs=1))
            # warmup PSUM tile shares the wave slots (transient).
            # ~10 warmups cover the ~6us preamble-to-first-chunk window.
            _warmup_pe(nc, warm_pool, ps, 10, tag="ps")

            # Hoist ALL input loads (both experts) to the front of the
            # sync (xg) / scalar (w) queues, ahead of any output store,
            # so expert 1's loads are never stuck behind expert 0's
            # stores in queue order. EPC tiles fit the pools exactly.
            assert EPC <= 2
            xg_ts, w_ts = [], []
            for e in range(EPC):
                xg_t = xg_pool.tile([P, KT, C], BF16, tag="xg",
                                    name=f"xg{e}")
                w_t = w_pool.tile([P, KT, DOUT], BF16, tag="w",
                                  name=f"w{e}")
                xg_ts.append(xg_t)
                w_ts.append(w_t)
                for k in range(KT):
                    lo, hi = k * P, (k + 1) * P
                    if e == 0 and k == 0:
                        # split only the very first chunk so the first
                        # matmuls start sooner after the DMA engines
                        # wake up (more splits would throttle the fill:
                        # descriptor issue costs ~600ns each)
                        half = 4 * P
                        nc.sync.dma_start(xg_t[:, 0, :half],
                                          xgT[0, :P, :half])
                        nc.sync.dma_start(xg_t[:, 0, half:],
                                          xgT[0, :P, half:])
                        nc.scalar.dma_start(w_t[:, 0], wexp[0, :P, :])
                    else:
                        nc.sync.dma_start(xg_t[:, k], xgT[e, lo:hi, :])
                        nc.scalar.dma_start(w_t[:, k], wexp[e, lo:hi, :])

            for e in range(EPC):
                xg_t, w_t = xg_ts[e], w_ts[e]
                # k-outer waves of concurrent PSUM groups, m-major.
                # The first wave of expert 0 takes all 8 banks: its
                # ~14us of matmul work hides the input-DMA fill and
                # keeps the PE busy so the clock ramp completes early.
                groups = [(m, n) for m in range(MT) for n in range(NT)]
                w0 = 0
                first = (e == 0)
                while w0 < len(groups):
                    wave = groups[w0:w0 + (8 if first else 4)]
                    w0 += len(wave)
                    first = False
                    pss = {g: ps.tile([P, NF], F32, tag="ps",
                                      name=f"ps_{e}_{g[0]}_{g[1]}")
                           for g in wave}
                    for k in range(KT):
                        for (m, n) in wave:
                            nc.tensor.matmul(
                                pss[(m, n)][:],
                                xg_t[:, k, m * P:(m + 1) * P],
                                w_t[:, k, n * NF:(n + 1) * NF],
                                start=(k == 0),
                                stop=(k == KT - 1),
                            )
                    # evict each finished group on the Vector engine
                    # and store right away, alternating store queues by
                    # output half so neither backs up
                    for (m, n) in wave:
                        dst = yout[e, m * P:(m + 1) * P,
                                   n * NF:(n + 1) * NF]
                        ot = out_pool.tile([P, NF], F32, tag="out",
                                           name=f"out_{e}_{m}_{n}")
                        nc.vector.tensor_copy(ot[:], pss[(m, n)][:])
                        eng = nc.sync if n == 0 else nc.scalar
                        eng.dma_start(dst, ot[:])
    nc.compile()
    _cache[key] = nc
    return nc


def _run(nc, in_maps):
    kw = {}
    if TRACE:
        kw["trace"] = True
    res = run_bass_kernel_spmd(nc, in_maps, list(range(NCORES)), **kw)
    if TRACE:
        LAST_EXEC_NS.append(res.exec_time_ns)
        LAST_RESULTS["last"] = res
    return res.results


def kernel(x, gate_w, gate_b, expert_w, expert_b, topk):
    x = np.ascontiguousarray(np.asarray(x, dtype=np.float32))
    gate_w = np.asarray(gate_w, dtype=np.float32)
    gate_b = np.asarray(gate_b, dtype=np.float32)
    expert_w = np.asarray(expert_w, dtype=np.float32)
    expert_b = np.asarray(expert_b, dtype=np.float32)
    topk = int(topk)

    B, DIN = x.shape
    E, _, DOUT = expert_w.shape
    assert B % P == 0 and DIN % P == 0
    EPC = E // NCORES
    assert EPC * NCORES == E

    # ---- host: gating (softmax + top-k) in float64 ----
    # Exact relative to the f32 reference: top-k margins (~1e-4 min)
    # dwarf the ~1e-5 f32 summation noise, so selection matches, and
    # the f64 probabilities are tighter than the reference's own f32.
    logits = x.astype(np.float64) @ gate_w.astype(np.float64).T \
        + gate_b.astype(np.float64)
    if topk < E:
        kth = np.partition(logits, E - topk, axis=1)[:, E - topk]
        mask = logits >= kth[:, None]
    else:
        mask = np.ones_like(logits, dtype=bool)
    z = np.exp(logits - logits.max(axis=1, keepdims=True))
    probs = z / z.sum(axis=1, keepdims=True)
    wfull = np.where(mask, probs, 0.0).astype(np.float32)

    # ---- host: routing bookkeeping + gather (pre-scaled, bf16) ----
    toks = [np.nonzero(wfull[:, e])[0] for e in range(E)]
    maxcnt = max(1, max(len(t) for t in toks))
    C = ((maxcnt + P - 1) // P) * P

    nc = _build_expert(C, DIN, DOUT, EPC)
    in_maps = []
    for c in range(NCORES):
        xgT = np.zeros((EPC, DIN, C), NPBF16)
        for j in range(EPC):
            e = EPC * c + j
            t = toks[e]
            xs = x[t] * wfull[t, e][:, None]      # gate-scaled tokens
            xgT[j, :, :len(t)] = xs.T.astype(NPBF16)
        in_maps.append({"xgT": xgT,
                        "wexp": expert_w[EPC * c:EPC * (c + 1)]
                        .astype(NPBF16)})
    r = _run(nc, in_maps)

    # ---- host: scatter-add compact outputs (unshard) ----
    y = np.zeros((B, DOUT), np.float32)
    for c in range(NCORES):
        yo = np.asarray(r[c]["yout"], dtype=np.float32)
        for j in range(EPC):
            e = EPC * c + j
            t = toks[e]
            y[t] += yo[j, :len(t)]
    if np.any(expert_b):
        for e in range(E):
            t = toks[e]
            y[t] += wfull[t, e][:, None] * expert_b[e][None, :]
    return y
